# revision 1
# baseline (speedup 1.0000x reference)
"""SAGEConv (max aggregation) + log_softmax, distributed over 8 NeuronCores.

Strategy: partition the nodes across the 8 cores (12500 dst nodes each, the
sharding hint's graph partitioning).  The per-edge gather x[src] runs on the
DMA engines via the tuned SWDGE gather instruction (InstDMAGatherAnt), which
takes int16 row indices — so the 100k-row feature table is split into 4
windows of 25000 rows, each staged with a -FLT_MAX filler row (max-identity
padding) and an all-zeros row (empty-aggregation semantics).

Level 1 (per window): each core's dst nodes are sorted by window in-degree
(descending) and packed into bands of 128; band b gets a uniform slot count
C = the band's max window-degree across all cores (SPMD uniformity), each
dst's slots holding its window sources padded with the filler row.  One
dma_gather per superblock of bands lands [dst x slot x feat] tiles in SBUF;
a strided vector reduce_max produces per-dst window partials, stored to a
DRAM staging buffer in window-band order.

Level 2: for each band of the final (total-degree-sorted) order, the 4
window partials are re-gathered by position (int16-safe: only 12545 rows)
and max-combined.  A tensor-engine transpose + PSUM-accumulated matmuls
apply lin_l/lin_r + bias, and a fused max/exp/sum/log pass forms
log_softmax.  The host un-permutes per-core outputs back to node order.
"""

import sys

import numpy as np

sys.path.insert(0, "/opt/trn_rl_repo")

from concourse import bacc, mybir, tile  # noqa: E402
from concourse.masks import make_identity  # noqa: E402

F32 = mybir.dt.float32
I16 = mybir.dt.int16

N_CORES = 8
P = 128
FILL = np.float32(-3.0e38)
S_MAX = 32  # max gather slots per level-1 superblock
NB1_MAX = 12  # max bands per level-1 superblock
NB2_MAX = 8  # max bands per level-2 superblock


def build_program(meta, iters=1, mode="full"):
    # mode: "full" | "l1g" (L1 gathers only) | "l1gr" (+reduces) |
    #        "l1" (+stores) | "l2" (L2 only)
    do_l1 = mode in ("full", "l1g", "l1gr", "l1")
    do_l1_reduce = mode in ("full", "l1gr", "l1")
    do_l1_store = mode in ("full", "l1")
    do_l2 = mode in ("full", "l2")
    d = meta["d_in"]
    do = meta["d_out"]
    nb = meta["nb"]
    ndst_pad = meta["ndst_pad"]
    nw = meta["nw"]
    wrows = meta["wrows"]
    l1 = meta["l1"]  # per window: list of sb dicts {o0, nslots, bands:[(b,o,C)]}
    idx1_cols = meta["idx1_cols"]  # per window
    l2 = meta["l2"]  # list of (b0, nbands)

    nc = bacc.Bacc(num_swdge_queues=4, dynamic_dma_scratch_size=32768)
    xw_t = [
        nc.declare_dram_parameter(f"xw{w}", [wrows, d], F32, isOutput=False)
        for w in range(nw)
    ]
    idx1_t = [
        nc.declare_dram_parameter(
            f"idx1_{w}", [P, max(idx1_cols[w], 1)], I16, isOutput=False
        )
        for w in range(nw)
    ]
    idx2_t = [
        nc.declare_dram_parameter(f"idx2_{w}", [P, 8 * nb], I16, isOutput=False)
        for w in range(nw)
    ]
    xopT_t = nc.declare_dram_parameter("xopT", [d, ndst_pad], F32, isOutput=False)
    wlT_t = nc.declare_dram_parameter("wlT", [d, do], F32, isOutput=False)
    wrT_t = nc.declare_dram_parameter("wrT", [d, do], F32, isOutput=False)
    bl_t = nc.declare_dram_parameter("bl", [1, do], F32, isOutput=False)
    y_t = nc.declare_dram_parameter("y", [ndst_pad, do], F32, isOutput=True)

    import contextlib

    with tile.TileContext(nc) as tc:
        with (
            tc.tile_pool(name="const", bufs=1) as cp,
            tc.tile_pool(name="dram", bufs=1, space="DRAM") as dp,
            tc.tile_pool(name="ps", bufs=2, space="PSUM") as pp,
            tc.tile_pool(name="l1", bufs=3) as wp1,
            tc.tile_pool(name="l2", bufs=2) as wp2,
        ):
            aggw_t = [
                dp.tile([ndst_pad + 1, d], F32, name=f"aggw{w}") for w in range(nw)
            ]

            wl_sb = cp.tile([d, do], F32)
            nc.sync.dma_start(out=wl_sb[:], in_=wlT_t[:])
            wr_sb = cp.tile([d, do], F32)
            nc.sync.dma_start(out=wr_sb[:], in_=wrT_t[:])
            bl_sb = cp.tile([1, do], F32)
            nc.sync.dma_start(out=bl_sb[:], in_=bl_t[:])
            ones_sb = cp.tile([1, P], F32)
            nc.vector.memset(ones_sb[:], 1.0)
            ident = cp.tile([P, P], F32)
            make_identity(nc, ident[:])
            fin_all = cp.tile([P, nb * do], F32)
            # FILL row of each aggw staging buffer (read by level-2 for
            # window-degree-0 dsts)
            fill_sb = cp.tile([1, d], F32)
            nc.vector.memset(fill_sb[:], float(FILL))
            for w in range(nw):
                nc.sync.dma_start(
                    out=aggw_t[w][ndst_pad : ndst_pad + 1, :], in_=fill_sb[:]
                )

            # ---- optional in-NEFF repeat loop for benchmarking ----
            loop_cm = tc.For_i(0, iters, 1) if iters > 1 else contextlib.nullcontext()
            with loop_cm:
                # ---- level 1: per-window gather + band reduce -> aggw ----
                wp = wp1
                qn = 0
                for w in range(nw if do_l1 else 0):
                        for sb in l1[w]:
                            ns = sb["nslots"]
                            bands = sb["bands"]
                            nbs = len(bands)
                            o0 = sb["o0"]
                            ncols = 8 * ns
                            idx_sb = wp.tile([P, 8 * S_MAX], I16, tag="idx")
                            nc.sync.dma_start(
                                out=idx_sb[:, :ncols],
                                in_=idx1_t[w][:, 8 * o0 : 8 * o0 + ncols],
                            )
                            gt = wp.tile([P, S_MAX * d], F32, tag="gt")
                            nc.gpsimd.dma_gather(
                                gt[:, : ns * d].rearrange("p (s f) -> p s f", f=d),
                                xw_t[w][:, :],
                                idx_sb[:, :ncols],
                                128 * ns,
                                128 * ns,
                                d,
                                single_packet=False,
                                queue_num=qn % 4,
                            )
                            qn += 1
                            assert nbs <= NB1_MAX
                            if not do_l1_reduce:
                                continue
                            aggsb = wp.tile([P, NB1_MAX * d], F32, tag="aggsb")
                            for k, (b, o, c) in enumerate(bands):
                                nc.vector.reduce_max(
                                    out=aggsb[:, k * d : (k + 1) * d],
                                    in_=gt[:, o * d : (o + c) * d].rearrange(
                                        "p (c f) -> p f c", f=d
                                    ),
                                    axis=mybir.AxisListType.X,
                                )
                            b0 = bands[0][0]
                            if do_l1_store:
                                nc.sync.dma_start(
                                    out=aggw_t[w][
                                        b0 * P : (b0 + nbs) * P, :
                                    ].rearrange("(t p) f -> p t f", p=P),
                                    in_=aggsb[:, : nbs * d].rearrange(
                                        "p (t f) -> p t f", f=d
                                    ),
                                )

                # ---- level 2: regather partials, combine, mm + softmax ----
                wp = wp2
                for b0, nbs in (l2 if do_l2 else []):
                    xop_sb = wp.tile([d, NB2_MAX * P], F32, tag="xop")
                    nc.sync.dma_start(
                        out=xop_sb[:, : nbs * P],
                        in_=xopT_t[:, b0 * P : (b0 + nbs) * P],
                    )
                    g2 = []
                    for w in range(nw):
                        ncols = 8 * nbs
                        idx_sb = wp.tile([P, 8 * NB2_MAX], I16, tag=f"idx2_{w}")
                        nc.sync.dma_start(
                            out=idx_sb[:, :ncols],
                            in_=idx2_t[w][:, 8 * b0 : 8 * b0 + ncols],
                        )
                        g = wp.tile([P, NB2_MAX * d], F32, tag=f"g2_{w}")
                        nc.gpsimd.dma_gather(
                            g[:, : nbs * d].rearrange("p (s f) -> p s f", f=d),
                            aggw_t[w][:, :],
                            idx_sb[:, :ncols],
                            128 * nbs,
                            128 * nbs,
                            d,
                            single_packet=False,
                            queue_num=w % 4,
                        )
                        g2.append(g)
                    agg_s = wp.tile([P, NB2_MAX * d], F32, tag="agg_s")
                    if nw == 1:
                        agg_s = g2[0]
                    elif nw == 2:
                        nc.vector.tensor_tensor(
                            out=agg_s[:, : nbs * d],
                            in0=g2[0][:, : nbs * d],
                            in1=g2[1][:, : nbs * d],
                            op=mybir.AluOpType.max,
                        )
                    else:
                        t01 = wp.tile([P, NB2_MAX * d], F32, tag="t01")
                        nc.vector.tensor_tensor(
                            out=t01[:, : nbs * d],
                            in0=g2[0][:, : nbs * d],
                            in1=g2[1][:, : nbs * d],
                            op=mybir.AluOpType.max,
                        )
                        if nw == 3:
                            nc.vector.tensor_tensor(
                                out=agg_s[:, : nbs * d],
                                in0=t01[:, : nbs * d],
                                in1=g2[2][:, : nbs * d],
                                op=mybir.AluOpType.max,
                            )
                        else:
                            t23 = wp.tile([P, NB2_MAX * d], F32, tag="t23")
                            nc.vector.tensor_tensor(
                                out=t23[:, : nbs * d],
                                in0=g2[2][:, : nbs * d],
                                in1=g2[3][:, : nbs * d],
                                op=mybir.AluOpType.max,
                            )
                            nc.vector.tensor_tensor(
                                out=agg_s[:, : nbs * d],
                                in0=t01[:, : nbs * d],
                                in1=t23[:, : nbs * d],
                                op=mybir.AluOpType.max,
                            )

                    for t in range(nbs):
                        b = b0 + t
                        aggT_p = pp.tile([P, d], F32, tag="tp")
                        nc.tensor.transpose(
                            out=aggT_p[:],
                            in_=agg_s[:, t * d : (t + 1) * d],
                            identity=ident[:],
                        )
                        aggT = wp.tile([P, d], F32, tag="aggT")
                        nc.vector.tensor_copy(out=aggT[:], in_=aggT_p[:])

                        op_p = pp.tile([P, do], F32, tag="op")
                        nc.tensor.matmul(
                            out=op_p[:],
                            lhsT=aggT[:],
                            rhs=wl_sb[:],
                            start=True,
                            stop=False,
                        )
                        nc.tensor.matmul(
                            out=op_p[:],
                            lhsT=xop_sb[:, t * P : (t + 1) * P],
                            rhs=wr_sb[:],
                            start=False,
                            stop=False,
                        )
                        nc.tensor.matmul(
                            out=op_p[:],
                            lhsT=ones_sb[:1, :],
                            rhs=bl_sb[:1, :],
                            start=False,
                            stop=True,
                        )

                        negm = wp.tile([P, 1], F32, tag="negm")
                        nc.vector.reduce_max(
                            out=negm[:],
                            in_=op_p[:],
                            axis=mybir.AxisListType.X,
                            negate=True,
                        )
                        e = wp.tile([P, do], F32, tag="e")
                        s = wp.tile([P, 1], F32, tag="s")
                        nc.scalar.activation(
                            out=e[:],
                            in_=op_p[:],
                            func=mybir.ActivationFunctionType.Exp,
                            bias=negm[:],
                            accum_out=s[:],
                        )
                        ls = wp.tile([P, 1], F32, tag="ls")
                        nc.scalar.activation(
                            out=ls[:], in_=s[:], func=mybir.ActivationFunctionType.Ln
                        )
                        shift = wp.tile([P, 1], F32, tag="shift")
                        nc.vector.tensor_tensor(
                            out=shift[:],
                            in0=negm[:],
                            in1=ls[:],
                            op=mybir.AluOpType.subtract,
                        )
                        nc.vector.tensor_tensor(
                            out=fin_all[:, b * do : (b + 1) * do],
                            in0=op_p[:],
                            in1=shift[:].to_broadcast([P, do]),
                            op=mybir.AluOpType.add,
                        )
                # one store for all bands: y[b*128+p, c] <- fin_all[p, b*do+c]
                if do_l2:
                    nc.sync.dma_start(
                        out=y_t[:, :].rearrange("(b p) c -> p b c", p=P),
                        in_=fin_all[:].rearrange("p (b c) -> p b c", c=do),
                    )
    nc.compile()
    return nc


def prepare(x, edge_index, W_l, b_l, W_r, n_cores=N_CORES, window_rows=25000):
    x = np.ascontiguousarray(np.asarray(x, dtype=np.float32))
    n, d = x.shape
    do = W_l.shape[0]
    src = np.asarray(edge_index[0], dtype=np.int64)
    dst = np.asarray(edge_index[1], dtype=np.int64)

    # drop duplicate (src,dst) pairs: max-aggregation is idempotent, and
    # every removed edge is one fewer SWDGE descriptor (the bottleneck)
    ekey = dst * np.int64(n) + src
    ekey = np.unique(ekey)
    dst = ekey // n
    src = ekey % n

    npc = (n + n_cores - 1) // n_cores
    nb = (npc + P - 1) // P
    ndst_pad = nb * P
    nw = (n + window_rows - 1) // window_rows
    wrows = window_rows + 2
    w_fill = window_rows  # local index of -FLT_MAX row
    w_zero = window_rows + 1  # local index of zeros row

    total_deg = np.bincount(dst, minlength=n).astype(np.int64)

    # window data + CSR
    xw = []
    deg_w = []
    srcs_w = []
    ptr_w = []
    for w in range(nw):
        lo, hi = w * window_rows, min((w + 1) * window_rows, n)
        arr = np.zeros((wrows, d), dtype=np.float32)
        arr[: hi - lo] = x[lo:hi]
        arr[w_fill] = FILL
        # arr[w_zero] stays zeros
        xw.append(arr)
        m = (src >= lo) & (src < hi)
        dw = dst[m]
        sw = src[m] - lo
        dg = np.bincount(dw, minlength=n).astype(np.int64)
        eo = np.argsort(dw, kind="stable")
        deg_w.append(dg)
        srcs_w.append(sw[eo])
        pt = np.zeros(n + 1, dtype=np.int64)
        np.cumsum(dg, out=pt[1:])
        ptr_w.append(pt)

    # per-core slot-indexed orderings
    # slot s in [0, ndst_pad): node id ids_ext[s] (or -1 for pads)
    ids_ext_all = []
    keyF_all = []
    keyW_all = []  # [core][window][slot]
    for c in range(n_cores):
        ids = np.arange(c * npc, min((c + 1) * npc, n))
        ids_ext = np.full(ndst_pad, -1, dtype=np.int64)
        ids_ext[: len(ids)] = ids
        ids_ext_all.append(ids_ext)
        kf = np.full(ndst_pad, -1, dtype=np.int64)
        kf[: len(ids)] = total_deg[ids]
        keyF_all.append(kf)
        kws = []
        for w in range(nw):
            kw = np.zeros(ndst_pad, dtype=np.int64)
            kw[: len(ids)] = deg_w[w][ids]
            if w == 0:
                # total-degree-0 reals and pads get one ZERO-row slot
                kw[: len(ids)][total_deg[ids] == 0] = 1
                kw[len(ids) :] = 1
            kws.append(kw)
        keyW_all.append(kws)

    orderF = [np.argsort(-keyF_all[c], kind="stable") for c in range(n_cores)]
    orderW = [
        [np.argsort(-keyW_all[c][w], kind="stable") for w in range(nw)]
        for c in range(n_cores)
    ]

    # global per-band slot counts, level-1
    cs1 = []  # [window][band]
    for w in range(nw):
        cs = []
        for b in range(nb):
            cs.append(
                int(
                    max(
                        keyW_all[c][w][orderW[c][w][b * P]] for c in range(n_cores)
                    )
                )
            )
        cs1.append(cs)

    # superblock packing (bands with C>0 only)
    l1 = []
    idx1_cols = []
    for w in range(nw):
        sbs = []
        cur = None
        o_glob = 0
        for b in range(nb):
            c = cs1[w][b]
            if c == 0:
                continue
            if (
                cur is None
                or cur["nslots"] + c > S_MAX
                or len(cur["bands"]) >= NB1_MAX
            ):
                cur = {"o0": o_glob, "nslots": 0, "bands": []}
                sbs.append(cur)
            cur["bands"].append((b, cur["nslots"], c))
            cur["nslots"] += c
            o_glob += c
        # bands within one store must be contiguous in aggw rows; since we
        # only skip trailing zero-C bands (degree-sorted), bands in each sb
        # are consecutive. Assert that.
        for sb in sbs:
            bs = [b for b, _, _ in sb["bands"]]
            assert bs == list(range(bs[0], bs[0] + len(bs)))
        l1.append(sbs)
        idx1_cols.append(8 * o_glob)

    # level-2 superblocks
    l2 = []
    b0 = 0
    while b0 < nb:
        nbs = min(NB2_MAX, nb - b0)
        l2.append((b0, nbs))
        b0 += nbs

    meta = {
        "n": n,
        "d_in": d,
        "d_out": do,
        "npc": npc,
        "nb": nb,
        "ndst_pad": ndst_pad,
        "nw": nw,
        "wrows": wrows,
        "l1": l1,
        "l2": l2,
        "idx1_cols": idx1_cols,
        "orders": orderF,
        "ids_ext": ids_ext_all,
    }

    # ---- build index arrays ----
    def pack16(flat):
        # idx position i -> row i%16, col i//16; the 16-partition block is
        # replicated 8x across the 128 partitions (one copy per GPSIMD core)
        m = len(flat)
        mc = (m + 15) // 16
        fl = np.zeros(mc * 16, dtype=np.int16)
        fl[:m] = flat
        block = np.ascontiguousarray(fl.reshape(mc, 16).T)
        return np.tile(block, (8, 1))

    in_maps = []
    wlT = np.ascontiguousarray(np.asarray(W_l, dtype=np.float32).T)
    wrT = np.ascontiguousarray(np.asarray(W_r, dtype=np.float32).T)
    bl = np.asarray(b_l, dtype=np.float32).reshape(1, do)

    for c in range(n_cores):
        ids_ext = ids_ext_all[c]
        im = {"wlT": wlT, "wrT": wrT, "bl": bl}
        for w in range(nw):
            im[f"xw{w}"] = xw[w]

        # level-1 indices
        for w in range(nw):
            ow = orderW[c][w]
            kw = keyW_all[c][w]
            dw = deg_w[w]
            pt = ptr_w[w]
            sw = srcs_w[w]
            segs = []
            for sb in l1[w]:
                ns = sb["nslots"]
                seg = np.full(128 * ns, w_fill, dtype=np.int64)
                for b, o, cbn in sb["bands"]:
                    slots = ow[b * P : (b + 1) * P]
                    nodes = ids_ext[slots]  # -1 for pads
                    real = nodes >= 0
                    dv = np.where(real, dw[np.maximum(nodes, 0)], 0)
                    base = np.where(real, pt[np.maximum(nodes, 0)], 0)
                    J = np.arange(cbn)[None, :]
                    gi = base[:, None] + np.minimum(J, np.maximum(dv - 1, 0)[:, None])
                    vals = np.where(
                        J < dv[:, None],
                        sw[np.minimum(gi, max(len(sw) - 1, 0))] if len(sw) else 0,
                        w_fill,
                    )
                    if w == 0:
                        zero_slot = (~real) | (
                            real & (total_deg[np.maximum(nodes, 0)] == 0)
                        )
                        vals[zero_slot, 0] = w_zero
                    # position i = (o + j)*128 + p
                    ii = ((o + J) * P + np.arange(P)[:, None]).ravel()
                    seg[ii] = vals.ravel()
                segs.append(seg)
            flat = np.concatenate(segs) if segs else np.zeros(0, dtype=np.int64)
            im[f"idx1_{w}"] = pack16(flat) if len(flat) else np.zeros(
                (P, 1), dtype=np.int16
            )

        # level-2 indices: for final position r, window position of that slot
        for w in range(nw):
            ow = orderW[c][w]
            posw = np.empty(ndst_pad, dtype=np.int64)
            posw[ow] = np.arange(ndst_pad)
            nb_active = sum(1 for cc in cs1[w] if cc > 0)
            pw = posw[orderF[c]]  # [ndst_pad] in final order
            pw = np.where(pw < nb_active * P, pw, ndst_pad)  # FILL row
            # idx position i = r (final slot) directly, since within sb
            # (b0,nbs) position t*128+p maps to r = (b0+t)*128+p
            im[f"idx2_{w}"] = pack16(pw)

        # x_own in final order, transposed
        oF = orderF[c]
        nodes = ids_ext[oF]
        xop = np.zeros((ndst_pad, d), dtype=np.float32)
        valid = nodes >= 0
        xop[valid] = x[nodes[valid]]
        im["xopT"] = np.ascontiguousarray(xop.T)

        in_maps.append(im)

    return in_maps, meta


def assemble(results, meta, n_cores=N_CORES):
    y = np.empty((meta["n"], meta["d_out"]), dtype=np.float32)
    for c in range(n_cores):
        oF = meta["orders"][c]
        nodes = meta["ids_ext"][c][oF]
        valid = nodes >= 0
        y[nodes[valid]] = results[c]["y"][valid]
    return y


def kernel(x, edge_index, W_l, b_l, W_r):
    from concourse.bass_utils import run_bass_kernel_spmd

    in_maps, meta = prepare(x, edge_index, W_l, b_l, W_r)
    nc = build_program(meta)
    res = run_bass_kernel_spmd(nc, in_maps, list(range(N_CORES)))
    return assemble(res.results, meta)


if __name__ == "__main__":
    rng = np.random.default_rng(0)
    n, e, d, do = 4000, 32000, 128, 64
    x = rng.standard_normal((n, d)).astype(np.float32)
    ei = rng.integers(0, n, size=(2, e))
    in_maps, meta = prepare(
        x,
        ei,
        rng.standard_normal((do, d), dtype=np.float32),
        np.zeros(do, np.float32),
        rng.standard_normal((do, d), dtype=np.float32),
        window_rows=1000,
    )
    print("nw:", meta["nw"], "l1 sbs:", [len(s) for s in meta["l1"]])

